# revision 1
# baseline (speedup 1.0000x reference)
"""v3 hybrid: per core, first NQA queries via dma_gather+DVE-mux (v2 path),
remaining NQB via per-column indirect DMA (v1 path). Pool runs both streams;
the DVE mux cost of the A-section hides under Pool, and A's lower per-query
Pool cost (8.6 vs 11.4 ns) cuts total Pool time."""

import numpy as np

P = 50
E = 2000
M = 64
F = 2_000_000
BASE = E + 2
PE = P * E
NCORES = 8
PART = 128
CHUNK = 1024
NQA = 44 * CHUNK        # 45_056 via dma_gather
NQB = 137 * PART        # 17_536 via indirect DMA
NP = NQA + NQB          # 62_592 (same as v1)
NTOT = NCORES * NP      # 500_736
RROWS = 2 * PE
RL = 65                 # int32 row: cnt + 64 win
RROWS8 = 2 * PE // 8
RL8 = 640


def _build_table(facts_idx: np.ndarray) -> np.ndarray:
    fp = facts_idx[:, 0].astype(np.int64)
    fs = facts_idx[:, 1].astype(np.int64)
    fo = facts_idx[:, 2].astype(np.int64)
    h = (fp * BASE + fs) * BASE + fo
    ho = np.argsort(h, kind="stable")
    fp, fs, fo = fp[ho], fs[ho], fo[ho]

    def csr(keys, vals):
        order = np.argsort(keys, kind="stable")
        svals = vals[order].astype(np.int32)
        counts = np.bincount(keys, minlength=PE)
        off = np.zeros(PE + 1, np.int64)
        np.cumsum(counts, out=off[1:])
        return svals, off

    def windows(svals, off):
        starts = off[:-1]
        cnt = np.minimum(off[1:] - starts, M).astype(np.int16)
        gi = np.minimum(starts[:, None] + np.arange(M, dtype=np.int64)[None, :], F - 1)
        return svals[gi].astype(np.int16), cnt

    ps_vals, ps_off = csr(fp * E + fs, fo)
    po_vals, po_off = csr(fp * E + fo, fs)
    w_ps, c_ps = windows(ps_vals, ps_off)   # [PE, 64], [PE]
    w_po, c_po = windows(po_vals, po_off)
    wins = np.concatenate([w_ps, w_po], axis=0)   # [2PE, 64] i16, r = dir*PE+key
    cnts = np.concatenate([c_ps, c_po], axis=0)   # [2PE] i16
    tab = np.zeros((RROWS8, RL8), np.int16)
    t3 = tab[:, : 8 * 72].reshape(RROWS8, 8, 72)
    t3[:, :, 0:64] = wins.reshape(RROWS8, 8, 64)
    t3[:, :, 64] = cnts.reshape(RROWS8, 8)
    return tab

def _permute_inputs(arr):
    """Return (W, N): W[p*S16+j]=arr[16j+p] (wrapped idx layout);
    N[p*C+cg]=arr[1024*(cg//8)+(cg%8)*128+p] (gather-slot layout)."""
    S16 = arr.shape[0] // 16
    C = arr.shape[0] // PART
    W = np.ascontiguousarray(arr.reshape(S16, 16).T).reshape(-1)
    p_idx = np.arange(PART)[:, None]
    cg = np.arange(C)[None, :]
    qmat = 1024 * (cg // 8) + (cg % 8) * 128 + p_idx
    N = np.ascontiguousarray(arr[qmat]).reshape(-1)
    return W, N



def _build_tab32(facts_idx):
    # int32 single-key rows (v1 table); reuse v2's CSR internals
    t16 = _build_table(facts_idx)  # [25000, 640] i16 (8-key rows, 72-groups)
    t3 = t16[:, : 8 * 72].reshape(RROWS8, 8, 72)
    tab = np.empty((RROWS, RL), np.int32)
    tab[:, 0] = t3[:, :, 64].reshape(-1)
    tab[:, 1:] = t3[:, :, 0:64].reshape(RROWS, 64)
    return tab


def _build_nc(nqa: int = NQA, nqb: int = NQB):
    import concourse.bacc as bacc
    import concourse.bass as bass
    import concourse.mybir as mybir
    import concourse.tile as tile

    nchunks = nqa // CHUNK
    S16 = nqa // 16
    CA = nqa // PART
    KB = nqb // PART
    kcb = 35  # v1-section chunk columns
    nc = bacc.Bacc("TRN2", target_bir_lowering=False, debug=False, num_devices=1)
    dt = mybir.dt
    Alu = mybir.AluOpType
    tab16 = nc.dram_tensor("tab16", [RROWS8, RL8], dt.int16, kind="ExternalInput")
    tab32 = nc.dram_tensor("tab32", [RROWS, RL], dt.int32, kind="ExternalInput")
    pw_d = nc.dram_tensor("pw", [nqa], dt.int32, kind="ExternalInput")
    bw_d = nc.dram_tensor("bw", [nqa], dt.int32, kind="ExternalInput")
    dw_d = nc.dram_tensor("dw", [nqa], dt.int32, kind="ExternalInput")
    pn_d = nc.dram_tensor("pn", [nqa], dt.int32, kind="ExternalInput")
    bn_d = nc.dram_tensor("bn", [nqa], dt.int32, kind="ExternalInput")
    dn_d = nc.dram_tensor("dn", [nqa], dt.int32, kind="ExternalInput")
    pb_d = nc.dram_tensor("pb", [nqb], dt.int32, kind="ExternalInput")
    bb_d = nc.dram_tensor("bb", [nqb], dt.int32, kind="ExternalInput")
    db_d = nc.dram_tensor("db", [nqb], dt.int32, kind="ExternalInput")
    n_q = nqa + nqb
    cand = nc.dram_tensor("cand", [n_q, M], dt.int32, kind="ExternalOutput")
    valid = nc.dram_tensor("valid", [n_q, M], dt.uint8, kind="ExternalOutput")

    candA = cand[0:nqa, :].rearrange("(k c p) m -> p k c m", p=PART, c=8)
    validA = valid[0:nqa, :].rearrange("(k c p) m -> p k c m", p=PART, c=8)
    candB = cand[nqa : nqa + nqb, :].rearrange("(p k) m -> p (k m)", p=PART)
    validB = valid[nqa : nqa + nqb, :].rearrange("(p k) m -> p (k m)", p=PART)

    with tile.TileContext(nc) as tc:
        with (
            tc.tile_pool(name="qp", bufs=1) as qp,
            tc.tile_pool(name="gp", bufs=5) as gp,
            tc.tile_pool(name="cp", bufs=4) as cp,
            tc.tile_pool(name="vp", bufs=4) as vp,
            tc.tile_pool(name="bp", bufs=3) as bp,
            tc.tile_pool(name="bvp", bufs=3) as bvp,
        ):
            # ======== B-section setup (v1 path) ========
            iota_t = qp.tile([PART, M], dt.int32)
            nc.gpsimd.iota(iota_t[:], pattern=[[1, M]], base=0, channel_multiplier=0)
            pB = qp.tile([PART, KB], dt.int32)
            bB = qp.tile([PART, KB], dt.int32)
            dB = qp.tile([PART, KB], dt.int32)
            idxB = qp.tile([PART, KB], dt.int32)
            nc.sync.dma_start(out=pB[:], in_=pb_d[:].rearrange("(p k) -> p k", p=PART))
            nc.sync.dma_start(out=bB[:], in_=bb_d[:].rearrange("(p k) -> p k", p=PART))
            nc.sync.dma_start(out=dB[:], in_=db_d[:].rearrange("(p k) -> p k", p=PART))
            nc.vector.tensor_scalar_mul(idxB[:], pB[:], E)
            nc.vector.tensor_tensor(out=idxB[:], in0=idxB[:], in1=bB[:], op=Alu.add)
            nc.vector.tensor_scalar_mul(dB[:], dB[:], PE)
            nc.vector.tensor_tensor(out=idxB[:], in0=idxB[:], in1=dB[:], op=Alu.add)
            iotaB_b = iota_t[:].rearrange("p (k m) -> p k m", k=1).to_broadcast(
                [PART, kcb, M]
            )

            # ======== A-section setup (v2 path) ========
            pw = qp.tile([16, S16], dt.int32)
            bw = qp.tile([16, S16], dt.int32)
            dw = qp.tile([16, S16], dt.int32)
            nc.sync.dma_start(out=pw[:], in_=pw_d[:].rearrange("(p c) -> p c", p=16))
            nc.sync.dma_start(out=bw[:], in_=bw_d[:].rearrange("(p c) -> p c", p=16))
            nc.sync.dma_start(out=dw[:], in_=dw_d[:].rearrange("(p c) -> p c", p=16))
            rw = qp.tile([16, S16], dt.int32)
            nc.vector.tensor_scalar_mul(rw[:], pw[:], E)
            nc.vector.tensor_tensor(out=rw[:], in0=rw[:], in1=bw[:], op=Alu.add)
            nc.vector.tensor_scalar_mul(dw[:], dw[:], PE)
            nc.vector.tensor_tensor(out=rw[:], in0=rw[:], in1=dw[:], op=Alu.add)
            row32 = qp.tile([16, S16], dt.int32)
            nc.vector.tensor_scalar(
                out=row32[:], in0=rw[:], scalar1=3, scalar2=None,
                op0=Alu.logical_shift_right,
            )
            row16 = qp.tile([16, S16], dt.int16)
            nc.vector.tensor_copy(row16[:], row32[:])
            idxr = qp.tile([PART, S16], dt.int16)
            for gidx in range(8):
                nc.sync.dma_start(out=idxr[16 * gidx : 16 * gidx + 16, :], in_=row16[:])

            p2 = qp.tile([PART, CA], dt.int32)
            b2 = qp.tile([PART, CA], dt.int32)
            d2 = qp.tile([PART, CA], dt.int32)
            nc.sync.dma_start(out=p2[:], in_=pn_d[:].rearrange("(p c) -> p c", p=PART))
            nc.sync.dma_start(out=b2[:], in_=bn_d[:].rearrange("(p c) -> p c", p=PART))
            nc.sync.dma_start(out=d2[:], in_=dn_d[:].rearrange("(p c) -> p c", p=PART))
            r2 = qp.tile([PART, CA], dt.int32)
            nc.vector.tensor_scalar_mul(r2[:], p2[:], E)
            nc.vector.tensor_tensor(out=r2[:], in0=r2[:], in1=b2[:], op=Alu.add)
            nc.vector.tensor_scalar_mul(d2[:], d2[:], PE)
            nc.vector.tensor_tensor(out=r2[:], in0=r2[:], in1=d2[:], op=Alu.add)
            sub = qp.tile([PART, CA], dt.int32)
            nc.vector.tensor_scalar(
                out=sub[:], in0=r2[:], scalar1=7, scalar2=None, op0=Alu.bitwise_and
            )
            msk = []
            for j in range(1, 8):
                m = qp.tile([PART, CA], dt.int32, tag=f"m{j}")
                nc.vector.tensor_scalar(
                    out=m[:], in0=sub[:], scalar1=j, scalar2=None, op0=Alu.is_equal
                )
                msk.append(m)
            iota_b = iota_t[:].rearrange("p (c m) -> p c m", c=1).to_broadcast(
                [PART, 8, M]
            )

            # ======== interleaved main loops ========
            nB_chunks = (KB + kcb - 1) // kcb
            b_cols = list(range(KB))
            b_chunks = [
                (ci * kcb, min(kcb, KB - ci * kcb)) for ci in range(nB_chunks)
            ]
            bi = 0  # next B chunk to emit

            def emit_b_chunk():
                nonlocal bi
                if bi >= len(b_chunks):
                    return
                c0, cw = b_chunks[bi]
                bi += 1
                gB = bp.tile([PART, kcb * RL], dt.int32, tag="gB")
                gB3 = gB[:].rearrange("p (k c) -> p k c", c=RL)
                for kk in range(cw):
                    nc.gpsimd.indirect_dma_start(
                        out=gB3[:, kk, :],
                        out_offset=None,
                        in_=tab32[:, :],
                        in_offset=bass.IndirectOffsetOnAxis(
                            ap=idxB[:, c0 + kk : c0 + kk + 1], axis=0
                        ),
                    )
                nc.sync.dma_start(
                    out=candB[:, c0 * M : (c0 + cw) * M], in_=gB3[:, 0:cw, 1:RL]
                )
                vB = bvp.tile([PART, kcb * M], dt.uint8, tag="vB")
                vB3 = vB[:].rearrange("p (k m) -> p k m", m=M)
                cntB = gB3[:, 0:cw, 0:1].to_broadcast([PART, cw, M])
                ib = iotaB_b if cw == kcb else iota_t[:].rearrange(
                    "p (k m) -> p k m", k=1
                ).to_broadcast([PART, cw, M])
                nc.vector.tensor_tensor(
                    out=vB3[:, 0:cw, :], in0=cntB, in1=ib, op=Alu.is_gt
                )
                nc.sync.dma_start(
                    out=validB[:, c0 * M : (c0 + cw) * M], in_=vB[:, 0 : cw * M]
                )

            emit_b_chunk()
            emit_b_chunk()
            for k in range(nchunks):
                g = gp.tile([PART, 8 * RL8], dt.int16, tag="g")
                g3 = g[:].rearrange("p (c e) -> p c e", e=RL8)
                nc.gpsimd.dma_gather(
                    out_ap=g3,
                    in_ap=tab16[:, :],
                    idxs_ap=idxr[:, k * 64 : k * 64 + 64],
                    num_idxs=CHUNK,
                    num_idxs_reg=CHUNK,
                    elem_size=RL8,
                )
                if k % 18 == 9:
                    emit_b_chunk()
                mb = [
                    m[:, k * 8 : k * 8 + 8]
                    .rearrange("p (c o) -> p c o", o=1)
                    .to_broadcast([PART, 8, 72])
                    for m in msk
                ]
                c16 = cp.tile([PART, 8 * 80], dt.int16, tag="c16")
                c163 = c16[:].rearrange("p (c m) -> p c m", m=80)[:, :, 0:72]
                nc.vector.tensor_copy(c163, g3[:, :, 0:72])
                for j in range(1, 8):
                    nc.vector.copy_predicated(
                        c163, mb[j - 1], g3[:, :, j * 72 : (j + 1) * 72]
                    )
                c16v = c16[:].rearrange("p (c m) -> p c m", m=80)
                c32 = cp.tile([PART, 8 * M], dt.int32, tag="c32")
                nc.vector.tensor_copy(
                    c32[:].rearrange("p (c m) -> p c m", m=M), c16v[:, :, 0:M]
                )
                nc.sync.dma_start(
                    out=candA[:, k, :, :],
                    in_=c32[:].rearrange("p (c m) -> p c m", m=M),
                )
                cnt32 = cp.tile([PART, 8], dt.int32, tag="cnt")
                nc.vector.tensor_copy(cnt32[:], c16v[:, :, M : M + 1])
                v = vp.tile([PART, 8 * M], dt.uint8, tag="v")
                v3 = v[:].rearrange("p (c m) -> p c m", m=M)
                nc.vector.tensor_tensor(
                    out=v3,
                    in0=cnt32[:].rearrange("p (c o) -> p c o", o=1).to_broadcast(
                        [PART, 8, M]
                    ),
                    in1=iota_b,
                    op=Alu.is_gt,
                )
                nc.sync.dma_start(out=validA[:, k, :, :], in_=v3)
            while bi < len(b_chunks):
                emit_b_chunk()
    nc.compile()
    return nc


_NC_CACHE = None
LAST_RESULT = None


def kernel(facts_idx, preds, bound_args, direction):
    global _NC_CACHE, LAST_RESULT
    from concourse.bass_utils import run_bass_kernel_spmd

    facts_idx = np.asarray(facts_idx, dtype=np.int32)
    preds = np.asarray(preds, dtype=np.int32)
    bound_args = np.asarray(bound_args, dtype=np.int32)
    direction = np.asarray(direction, dtype=np.int32)

    tab16 = _build_table(facts_idx)
    tab32 = _build_tab32(facts_idx)

    n = preds.shape[0]
    pad = NTOT - n
    p_pad = np.pad(preds, (0, pad))
    b_pad = np.pad(bound_args, (0, pad))
    d_pad = np.pad(direction, (0, pad))

    if _NC_CACHE is None:
        _NC_CACHE = _build_nc()
    nc = _NC_CACHE

    in_maps = []
    for c in range(NCORES):
        qa = slice(c * NP, c * NP + NQA)
        qb = slice(c * NP + NQA, (c + 1) * NP)
        pw_, pn_ = _permute_inputs(p_pad[qa])
        bw_, bn_ = _permute_inputs(b_pad[qa])
        dw_, dn_ = _permute_inputs(d_pad[qa])
        in_maps.append({
            "tab16": tab16, "tab32": tab32,
            "pw": pw_, "bw": bw_, "dw": dw_,
            "pn": pn_, "bn": bn_, "dn": dn_,
            "pb": np.ascontiguousarray(p_pad[qb]),
            "bb": np.ascontiguousarray(b_pad[qb]),
            "db": np.ascontiguousarray(d_pad[qb]),
        })
    res = run_bass_kernel_spmd(nc, in_maps, core_ids=list(range(NCORES)))
    LAST_RESULT = res
    cand = np.concatenate([r["cand"] for r in res.results], axis=0)[:n]
    valid = np.concatenate([r["valid"] for r in res.results], axis=0)[:n]
    return cand, valid.astype(bool)



# revision 4
# speedup vs baseline: 6.2347x; 6.2347x over previous
"""v7: table-sharded one-hot matmul gather.

Table split into 1563 tiles of 128 rows; core c owns tiles [196c, 196(c+1)).
Host routes each query to the core owning its key's tile and assigns it a
(chunk, partition) slot; Q=384 slots per tile (3 chunks of 128), overflow
(max observed 392) goes to a 512-query spill handled by indirect DMA.

On device the per-core table slice (128 rows x 196*65 fp16, 25.5KB/part)
is SBUF-resident. Per chunk: onehot[row, slot] = (keyl[slot] == row) built
by DVE against a partition iota, then TensorE matmul
  out[slot, e] = sum_r onehot[r, slot] * tab[r, e]
lands rows in PSUM with slot = partition. Scalar engine drains values to
uint16 (host widens to int32), DVE computes valid = iota < cnt.
GpSimd only handles the 4-call spill, so the ~8ns/query descriptor
bottleneck of gather/indirect designs disappears.
"""

import numpy as np

P = 50
E = 2000
M = 64
F = 2_000_000
BASE = E + 2
PE = P * E
NCORES = 8
PART = 128
RROWS = 2 * PE            # 200000
TILES = 1563              # ceil(RROWS/128)
TILES_PAD = 1568          # NCORES*TPC, table padded to 200704 rows
TPC = 196                 # tiles per core (core 7: 191 real + 5 dummy)
Q = 384                   # slots per tile = 3 chunks of 128
CH = TPC * 3              # 588 chunks per core
RL = 65                   # row: 64 window values + count
NS = 512                  # spill slots per core
NSC = NS // PART          # spill indirect calls
SEP = 7                   # chunks per PSUM tile
GSEP = 6                  # septets per output batch
GCH = SEP * GSEP          # 28 chunks per batch
NG = CH // GCH            # 14 batches


def _build_rows(facts_idx: np.ndarray):
    """[200064, 65] windows+count table (int16 range values)."""
    fp = facts_idx[:, 0].astype(np.int64)
    fs = facts_idx[:, 1].astype(np.int64)
    fo = facts_idx[:, 2].astype(np.int64)
    h = (fp * BASE + fs) * BASE + fo
    ho = np.argsort(h, kind="stable")
    fp, fs, fo = fp[ho], fs[ho], fo[ho]

    def csr(keys, vals):
        order = np.argsort(keys, kind="stable")
        svals = vals[order].astype(np.int32)
        counts = np.bincount(keys, minlength=PE)
        off = np.zeros(PE + 1, np.int64)
        np.cumsum(counts, out=off[1:])
        return svals, off

    def windows(svals, off):
        starts = off[:-1]
        cnt = np.minimum(off[1:] - starts, M).astype(np.int16)
        gi = np.minimum(starts[:, None] + np.arange(M, dtype=np.int64)[None, :], F - 1)
        return svals[gi].astype(np.int16), cnt

    ps_vals, ps_off = csr(fp * E + fs, fo)
    po_vals, po_off = csr(fp * E + fo, fs)
    w_ps, c_ps = windows(ps_vals, ps_off)
    w_po, c_po = windows(po_vals, po_off)
    rows = np.zeros((TILES_PAD * PART, RL), np.int16)
    rows[:PE, :M] = w_ps
    rows[:PE, M] = c_ps
    rows[PE:RROWS, :M] = w_po
    rows[PE:RROWS, M] = c_po
    return rows


def _build_nc():
    import concourse.bacc as bacc
    import concourse.bass as bass
    import concourse.mybir as mybir
    import concourse.tile as tile

    nc = bacc.Bacc("TRN2", target_bir_lowering=False, debug=False, num_devices=1)
    dt = mybir.dt
    Alu = mybir.AluOpType

    tabT_d = nc.dram_tensor("tabT", [PART, TPC * RL], dt.float16, kind="ExternalInput")
    tab32_d = nc.dram_tensor("tab32", [RROWS, RL], dt.int32, kind="ExternalInput")
    oh_d = nc.dram_tensor("oh", [PART, CH * PART], dt.float8e4, kind="ExternalInput")
    skey_d = nc.dram_tensor("skey", [NS], dt.int32, kind="ExternalInput")
    cand_d = nc.dram_tensor("cand", [PART, CH * M], dt.uint16, kind="ExternalOutput")
    valid_d = nc.dram_tensor("valid", [PART, CH * M], dt.uint8, kind="ExternalOutput")
    candS_d = nc.dram_tensor("candS", [NS, M], dt.int32, kind="ExternalOutput")
    validS_d = nc.dram_tensor("validS", [NS, M], dt.uint8, kind="ExternalOutput")

    candS_r = candS_d[:, :].rearrange("(k p) m -> p k m", p=PART)
    validS_r = validS_d[:, :].rearrange("(k p) m -> p k m", p=PART)

    with tile.TileContext(nc) as tc:
        with (
            tc.tile_pool(name="qp", bufs=1) as qp,
            tc.tile_pool(name="kp", bufs=3) as kp,
            tc.tile_pool(name="op", bufs=3) as op,
            tc.tile_pool(name="pp", bufs=8, space="PSUM") as pp,
            tc.tile_pool(name="cp", bufs=3) as cp,
            tc.tile_pool(name="vp", bufs=3) as vp,
            tc.tile_pool(name="wp", bufs=3) as wp,
            tc.tile_pool(name="sp", bufs=2) as sp,
        ):
            tabt = qp.tile([PART, TPC * RL], dt.float16)
            nc.sync.dma_start(out=tabt[:], in_=tabT_d[:, :])
            iota64 = qp.tile([PART, M], dt.int32)
            nc.gpsimd.iota(iota64[:], pattern=[[1, M]], base=0, channel_multiplier=0)
            iota16 = qp.tile([PART, M], dt.int16)
            nc.vector.tensor_copy(iota16[:], iota64[:])
            iota16_w = qp.tile([PART, SEP * M], dt.int16)
            nc.vector.tensor_copy(
                iota16_w[:],
                iota16[:].rearrange("p (c m) -> p c m", c=1).to_broadcast(
                    [PART, SEP, M]
                ),
            )

            # ---- spill path (gpsimd is otherwise idle) ----
            skeyt = qp.tile([PART, NSC], dt.int32)
            nc.sync.dma_start(
                out=skeyt[:], in_=skey_d[:].rearrange("(k p) -> p k", p=PART)
            )
            for k in range(NSC):
                gS = sp.tile([PART, RL], dt.int32, tag="gS")
                # the scheduler doesn't track the indirect offset-AP read, so
                # chain skeyt(DMA) -> DVE -> gS(WAW) to order the gather after
                # the key load
                nc.vector.tensor_copy(gS[:, 0:NSC], skeyt[:])
                nc.gpsimd.indirect_dma_start(
                    out=gS[:],
                    out_offset=None,
                    in_=tab32_d[:, :],
                    in_offset=bass.IndirectOffsetOnAxis(ap=skeyt[:, k : k + 1], axis=0),
                )
                nc.sync.dma_start(out=candS_r[:, k, :], in_=gS[:, 0:M])
                vS = sp.tile([PART, M], dt.uint8, tag="vS")
                nc.vector.tensor_tensor(
                    out=vS[:],
                    in0=gS[:, M : M + 1].to_broadcast([PART, M]),
                    in1=iota64[:],
                    op=Alu.is_gt,
                )
                nc.sync.dma_start(out=validS_r[:, k, :], in_=vS[:])

            # ---- main loop ----
            for g in range(NG):
                ohg = kp.tile([PART, GCH * PART], dt.float8e4, tag="ohg")
                nc.gpsimd.dma_start(
                    out=ohg[:],
                    in_=oh_d[:, g * GCH * PART : (g + 1) * GCH * PART],
                )
                cg = cp.tile([PART, GCH * M], dt.uint16, tag="cg")
                vg = vp.tile([PART, GCH * M], dt.uint8, tag="vg")
                for s in range(GSEP):
                    pt = pp.tile([PART, SEP * RL], dt.float32, space="PSUM", tag="pt")
                    for j in range(SEP):
                        ch = (g * GSEP + s) * SEP + j
                        t = ch // 3
                        nc.tensor.matmul(
                            out=pt[:, j * RL : (j + 1) * RL],
                            lhsT=ohg[:, (s * SEP + j) * PART : (s * SEP + j + 1) * PART],
                            rhs=tabt[:, t * RL : (t + 1) * RL],
                            start=True,
                            stop=True,
                        )
                    pt3 = pt[:].rearrange("p (c e) -> p c e", e=RL)
                    cg_sl = cg[:, s * SEP * M : (s + 1) * SEP * M].rearrange(
                        "p (c m) -> p c m", m=M
                    )
                    if s == 3:
                        nc.vector.tensor_copy(cg_sl, pt3[:, :, 0:M])
                    else:
                        nc.scalar.copy(cg_sl, pt3[:, :, 0:M])
                    cnt16 = wp.tile([PART, SEP], dt.int16, tag="cnt")
                    nc.vector.tensor_copy(cnt16[:], pt3[:, :, M : M + 1])
                    nc.vector.tensor_tensor(
                        out=vg[:, s * SEP * M : (s + 1) * SEP * M].rearrange(
                            "p (c m) -> p c m", m=M
                        ),
                        in0=cnt16[:]
                        .rearrange("p (c o) -> p c o", o=1)
                        .to_broadcast([PART, SEP, M]),
                        in1=iota16_w[:].rearrange("p (c m) -> p c m", m=M),
                        op=Alu.is_gt,
                    )
                nc.sync.dma_start(
                    out=cand_d[:, g * GCH * M : (g + 1) * GCH * M], in_=cg[:]
                )
                nc.scalar.dma_start(
                    out=valid_d[:, g * GCH * M : (g + 1) * GCH * M], in_=vg[:]
                )
    nc.compile()
    return nc


_NC_CACHE = None
LAST_RESULT = None


def kernel(facts_idx, preds, bound_args, direction):
    global _NC_CACHE, LAST_RESULT
    from concourse.bass_utils import run_bass_kernel_spmd
    import ml_dtypes
    _f8 = ml_dtypes.float8_e4m3

    facts_idx = np.asarray(facts_idx, dtype=np.int32)
    preds = np.asarray(preds, dtype=np.int32)
    bound_args = np.asarray(bound_args, dtype=np.int32)
    direction = np.asarray(direction, dtype=np.int32)
    n = preds.shape[0]

    rows = _build_rows(facts_idx)                       # [200064, 65] i16
    tab32 = rows[:RROWS].astype(np.int32)               # spill table, same layout

    keys = (
        direction.astype(np.int64) * PE
        + preds.astype(np.int64) * E
        + bound_args.astype(np.int64)
    ).astype(np.int32)
    tiles = keys >> 7
    order = np.argsort(tiles, kind="stable")
    tsort = tiles[order]
    counts = np.bincount(tsort, minlength=TILES)
    starts = np.zeros(TILES, np.int64)
    np.cumsum(counts[:-1], out=starts[1:])
    occ = np.arange(n, dtype=np.int64) - starts[tsort]  # rank within tile

    core = tsort // TPC
    lt = tsort - core * TPC                             # local tile
    main = occ < Q
    chunk = lt * 3 + (occ >> 7)
    part = occ & 127

    oh_u8 = np.zeros((NCORES, PART, CH * PART), np.uint8)
    lrow = (keys[order[main]] & 127).astype(np.int64)
    flat = (
        core[main] * (PART * CH * PART)
        + lrow * (CH * PART)
        + chunk[main] * PART
        + part[main]
    ).astype(np.int64)
    oh_u8.reshape(-1)[flat] = 0x38  # fp8 e4m3 1.0

    skey = np.zeros((NCORES, NS), np.int32)
    sidx = np.zeros(n, np.int64)                        # spill position per sorted query
    if (~main).any():
        sp_core = core[~main]
        sp_occ = np.zeros(len(sp_core), np.int64)
        for c in range(NCORES):
            m = sp_core == c
            nsp = int(m.sum())
            assert nsp <= NS, f"spill overflow core {c}: {nsp}"
            sp_occ[m] = np.arange(nsp)
            skey[c, :nsp] = keys[order[~main]][m]
        sidx[~main] = sp_core * NS + sp_occ

    if _NC_CACHE is None:
        _NC_CACHE = _build_nc()
    nc = _NC_CACHE

    tabTs = []
    for c in range(NCORES):
        sl = rows[c * TPC * PART : (c + 1) * TPC * PART].reshape(TPC, PART, RL)
        tabTs.append(
            np.ascontiguousarray(sl.transpose(1, 0, 2).reshape(PART, TPC * RL)).astype(
                np.float16
            )
        )

    in_maps = [
        {
            "tabT": tabTs[c],
            "tab32": tab32,
            "oh": oh_u8[c].view(_f8),
            "skey": np.ascontiguousarray(skey[c]),
        }
        for c in range(NCORES)
    ]
    res = run_bass_kernel_spmd(nc, in_maps, core_ids=list(range(NCORES)))
    LAST_RESULT = res

    # ---- assemble ----
    candM = np.stack([r["cand"] for r in res.results])      # [8, 128, CH*M] u16
    validM = np.stack([r["valid"] for r in res.results])    # [8, 128, CH*M] u8
    candS = np.stack([r["candS"] for r in res.results])     # [8, NS, M] i32
    validS = np.stack([r["validS"] for r in res.results])

    cand = np.empty((n, M), np.int32)
    valid = np.empty((n, M), np.uint8)
    om = order[main]
    candM = candM.reshape(NCORES, PART, CH, M)
    validM = validM.reshape(NCORES, PART, CH, M)
    cand[om] = candM[core[main], part[main], chunk[main]].astype(np.int32)
    valid[om] = validM[core[main], part[main], chunk[main]]
    if (~main).any():
        osp = order[~main]
        si = sidx[~main]
        cand[osp] = candS.reshape(-1, M)[si]
        valid[osp] = validS.reshape(-1, M)[si]
    return cand, valid.astype(bool)
